# revision 6
# baseline (speedup 1.0000x reference)
"""Block-causal attention block (RMSnorm + QKV + frame-causal attention + proj)
on 8 TRN2 NeuronCores.

Sharding: sequence-parallel over the 8 frames — core i owns the 1024 queries of
frame i and processes KV blocks for frames 0..i (uniform SPMD program: all 16
half-blocks are processed on every core; future frames are killed by a
per-core additive bias of -1e30 before the exp, so they contribute exp() = 0
to both the numerator and denominator of the softmax).

Layouts are channel-first throughout ([C, seq] with C on partitions), which
makes every contraction a natural PE matmul with no transposes:
  k^T [C, kv]   = Wk' @ xn           (lhsT = wkT chunk, rhs = xn)
  v   [kv, C]   = xn^T @ Wv'^T       (lhsT = xn chunk,  rhs = wvT)
  S^T [kv, q]   = K @ Q^T            (lhsT = k^T chunk, rhs = q^T)
  O^T [C, q]    = V^T @ P^T          (lhsT = v chunk,   rhs = p^T)
  den [1, q]    = ones^T @ P^T       (lhsT = ones,      rhs = p^T)

Host-side folds: gamma*sqrt(C) into wq/wk/wv; bv through wp into the output
bias (softmax rows sum to 1); no max-subtraction in the softmax (scores here
are O(1); exp is safe and matches jax.nn.softmax exactly up to rounding).

All matmuls run in float32r (FP22) — full-rate on TRN2 with ~1e-4 accuracy.
"""

import sys

import numpy as np

sys.path.insert(0, "/opt/trn_rl_repo")

import concourse.bacc as bacc
import concourse.bass as bass  # noqa: F401
import concourse.tile as tile
from concourse import mybir
from concourse.bass_utils import run_bass_kernel_spmd

C = 512
CC = C // 128          # 4 channel chunks
F = 8                  # frames
HW = 1024              # tokens per frame
SEQ = F * HW           # 8192
S = 512                # kv columns processed per step
NSTEP = SEQ // S       # 16
Q = 1024               # queries per core (one frame)
QH = Q // S            # 2 query halves
KVC = SEQ // 128       # 64 kv 128-chunks
SCALE = 1.0 / float(np.sqrt(C))
NEG = -1.0e30

F32 = mybir.dt.float32
F32R = mybir.dt.float32r
Act = mybir.ActivationFunctionType

_cached = {}


def _build():
    if "nc" in _cached:
        return _cached["nc"]

    nc = bacc.Bacc()
    xq_d = nc.dram_tensor("xq", [C, Q], F32, kind="ExternalInput")
    xkv_d = nc.dram_tensor("xkv", [C, SEQ], F32, kind="ExternalInput")
    kvb_d = nc.dram_tensor("kvb", [128, KVC], F32, kind="ExternalInput")
    wq_d = nc.dram_tensor("wqT", [C, C], F32, kind="ExternalInput")
    wk_d = nc.dram_tensor("wkT", [C, C], F32, kind="ExternalInput")
    wv_d = nc.dram_tensor("wvT", [C, C], F32, kind="ExternalInput")
    wp_d = nc.dram_tensor("wpT", [C, C], F32, kind="ExternalInput")
    bq_d = nc.dram_tensor("bq", [C, 1], F32, kind="ExternalInput")
    bk_d = nc.dram_tensor("bk", [C, 1], F32, kind="ExternalInput")
    bvp_d = nc.dram_tensor("bvp", [C, 1], F32, kind="ExternalInput")
    out_d = nc.dram_tensor("out", [C, Q], F32, kind="ExternalOutput")

    with tile.TileContext(nc) as tc:
        with (
            tc.tile_pool(name="const", bufs=1) as const,
            tc.tile_pool(name="persist", bufs=1) as persist,
            tc.tile_pool(name="xload", bufs=2) as xload,
            tc.tile_pool(name="norm", bufs=2) as norm,
            tc.tile_pool(name="kv", bufs=2) as kvpool,
            tc.tile_pool(name="ppool", bufs=2) as ppool,
            tc.tile_pool(name="psum_s", bufs=2, space="PSUM") as psum_s,
            tc.tile_pool(name="psum_o", bufs=2, space="PSUM") as psum_o,
            tc.tile_pool(name="psum_den", bufs=1, space="PSUM") as psum_den,
        ):
            # ---- constants / weights (wq and wp share one slot: wp is only
            # needed after the last use of wq) ----
            wq_sb = const.tile([128, CC, C], F32R, tag="wqp", name="wq_sb")
            wk_sb = const.tile([128, CC, C], F32R, tag="wk", name="wk_sb")
            wv_sb = const.tile([128, CC, C], F32R, tag="wv", name="wv_sb")
            for w_sb, w_d in ((wq_sb, wq_d), (wk_sb, wk_d), (wv_sb, wv_d)):
                for ci in range(CC):
                    nc.sync.dma_start(
                        out=w_sb[:, ci, :],
                        in_=w_d[ci * 128:(ci + 1) * 128, :].bitcast(F32R),
                    )
            bq_sb = const.tile([128, CC], F32, tag="bq", name="bq_sb")
            bk_sb = const.tile([128, CC], F32, tag="bk", name="bk_sb")
            bvp_sb = const.tile([128, CC], F32, tag="bvp", name="bvp_sb")
            for b_sb, b_d in ((bq_sb, bq_d), (bk_sb, bk_d), (bvp_sb, bvp_d)):
                for ci in range(CC):
                    nc.sync.dma_start(
                        out=b_sb[:, ci:ci + 1],
                        in_=b_d[ci * 128:(ci + 1) * 128, :],
                    )
            kvb_sb = const.tile([128, KVC], F32, tag="kvb", name="kvb_sb")
            nc.sync.dma_start(out=kvb_sb[:], in_=kvb_d[:])
            ones_f = const.tile([128, 1], F32, tag="ones_f", name="ones_f")
            nc.vector.memset(ones_f[:], 1.0)
            ones_sb = const.tile([128, 1], F32R, tag="ones", name="ones_sb")
            nc.vector.tensor_copy(ones_sb[:], ones_f[:])

            # ---- persistent q-side tiles ----
            qT_sb = persist.tile([128, CC, Q], F32R, tag="qT", name="qT_sb")
            o_sb = persist.tile([128, CC, Q], F32, tag="o", name="o_sb")
            rdb = persist.tile([128, Q], F32, tag="rdb", name="rdb")
            den_ps = [
                psum_den.tile([1, S], F32, tag=f"den{qh}", name=f"den{qh}")
                for qh in range(QH)
            ]

            def load_norm(x_dram, col0):
                """DMA a [C, S] slab, compute per-column 1/l2norm, return
                xn = x * rnorm (fp32r, rounded) reusing one rotating slot."""
                xt = xload.tile([128, CC, S], F32, tag="xt", name="xt")
                for ci in range(CC):
                    nc.sync.dma_start(
                        out=xt[:, ci, :],
                        in_=x_dram[ci * 128:(ci + 1) * 128, col0:col0 + S],
                    )
                xn = norm.tile([128, CC, S], F32R, tag="xn", name="xn")
                ss_ps = psum_s.tile([1, S], F32, tag="s", name="ss_ps")
                for ci in range(CC):
                    nc.scalar.square(xn[:, ci, :], xt[:, ci, :])
                for ci in range(CC):
                    nc.tensor.matmul(
                        ss_ps[:], ones_sb[:], xn[:, ci, :],
                        start=(ci == 0), stop=(ci == CC - 1),
                    )
                rn = norm.tile([1, S], F32, tag="rn", name="rn")
                nc.scalar.sqrt(rn[:], ss_ps[:])
                nc.vector.tensor_scalar_max(rn[:], rn[:], 1.0e-12)
                nc.vector.reciprocal(rn[:], rn[:])
                rnb = norm.tile([128, S], F32, tag="rnb", name="rnb")
                nc.gpsimd.partition_broadcast(rnb[:], rn[:])
                for ci in range(CC):
                    nc.vector.tensor_mul(xn[:, ci, :], xt[:, ci, :], rnb[:])
                return xn

            # ---- Q path (once) ----
            for qh in range(QH):
                xn = load_norm(xq_d, qh * S)
                for co in range(CC):
                    q_ps = psum_o.tile([128, S], F32, tag="proj", name="q_ps")
                    for ci in range(CC):
                        nc.tensor.matmul(
                            q_ps[:],
                            wq_sb[:, ci, co * 128:(co + 1) * 128],
                            xn[:, ci, :],
                            start=(ci == 0), stop=(ci == CC - 1),
                        )
                    nc.scalar.activation(
                        qT_sb[:, co, qh * S:(qh + 1) * S], q_ps[:],
                        Act.Identity, bias=bq_sb[:, co:co + 1], scale=1.0,
                    )

            # wp loads into wq's slot once wq is no longer needed
            wp_sb = const.tile([128, CC, C], F32R, tag="wqp", name="wp_sb")
            for ci in range(CC):
                nc.sync.dma_start(
                    out=wp_sb[:, ci, :],
                    in_=wp_d[ci * 128:(ci + 1) * 128, :].bitcast(F32R),
                )

            # ---- KV steps ----
            for t in range(NSTEP):
                xn = load_norm(xkv_d, t * S)

                kT = kvpool.tile([128, CC, S], F32R, tag="kT", name="kT")
                for co in range(CC):
                    k_ps = psum_o.tile([128, S], F32, tag="proj", name="k_ps")
                    for ci in range(CC):
                        nc.tensor.matmul(
                            k_ps[:],
                            wk_sb[:, ci, co * 128:(co + 1) * 128],
                            xn[:, ci, :],
                            start=(ci == 0), stop=(ci == CC - 1),
                        )
                    nc.scalar.activation(
                        kT[:, co, :], k_ps[:],
                        Act.Identity, bias=bk_sb[:, co:co + 1], scale=1.0,
                    )

                v_sb = kvpool.tile([128, S // 128, C], F32R, tag="v", name="v_sb")
                for kp in range(S // 128):
                    v_ps = psum_o.tile([128, C], F32, tag="proj", name="v_ps")
                    for ci in range(CC):
                        nc.tensor.matmul(
                            v_ps[:],
                            xn[:, ci, kp * 128:(kp + 1) * 128],
                            wv_sb[:, ci, :],
                            start=(ci == 0), stop=(ci == CC - 1),
                        )
                    nc.vector.tensor_copy(v_sb[:, kp, :], v_ps[:])

                p_sb = ppool.tile([128, S // 128, Q], F32R, tag="p", name="p_sb")
                for kp in range(S // 128):
                    kvi = t * (S // 128) + kp
                    for qh in range(QH):
                        s_ps = psum_s.tile([128, S], F32, tag="s", name="s_ps")
                        for ci in range(CC):
                            nc.tensor.matmul(
                                s_ps[:],
                                kT[:, ci, kp * 128:(kp + 1) * 128],
                                qT_sb[:, ci, qh * S:(qh + 1) * S],
                                start=(ci == 0), stop=(ci == CC - 1),
                            )
                        nc.scalar.activation(
                            p_sb[:, kp, qh * S:(qh + 1) * S], s_ps[:],
                            Act.Exp, bias=kvb_sb[:, kvi:kvi + 1], scale=SCALE,
                        )
                        nc.tensor.matmul(
                            den_ps[qh][:], ones_sb[:],
                            p_sb[:, kp, qh * S:(qh + 1) * S],
                            start=(t == 0 and kp == 0),
                            stop=(t == NSTEP - 1 and kp == S // 128 - 1),
                        )

                for co in range(CC):
                    for qh in range(QH):
                        o_ps = psum_o.tile([128, S], F32, tag="o", name="o_ps")
                        for kp in range(S // 128):
                            nc.tensor.matmul(
                                o_ps[:],
                                v_sb[:, kp, co * 128:(co + 1) * 128],
                                p_sb[:, kp, qh * S:(qh + 1) * S],
                                start=(kp == 0), stop=(kp == S // 128 - 1),
                            )
                        if t == 0:
                            nc.vector.tensor_copy(
                                o_sb[:, co, qh * S:(qh + 1) * S], o_ps[:]
                            )
                        else:
                            nc.vector.tensor_add(
                                o_sb[:, co, qh * S:(qh + 1) * S],
                                o_sb[:, co, qh * S:(qh + 1) * S],
                                o_ps[:],
                            )

            # ---- finalize: normalize, project, residual ----
            for qh in range(QH):
                rd = norm.tile([1, S], F32, tag="rn", name="rd")
                nc.vector.reciprocal(rd[:], den_ps[qh][:])
                nc.gpsimd.partition_broadcast(rdb[:, qh * S:(qh + 1) * S], rd[:])
            # o_n := o * (1/den), rounded to fp32r (reuses a p-pool slot)
            on_sb = ppool.tile([128, CC, Q], F32R, tag="p", name="on_sb")
            for ci in range(CC):
                nc.vector.tensor_mul(on_sb[:, ci, :], o_sb[:, ci, :], rdb[:])
            for qh in range(QH):
                xr = xload.tile([128, CC, S], F32, tag="xt", name="xr")
                for ci in range(CC):
                    nc.sync.dma_start(
                        out=xr[:, ci, :],
                        in_=xq_d[ci * 128:(ci + 1) * 128, qh * S:(qh + 1) * S],
                    )
                for co in range(CC):
                    pr_ps = psum_o.tile([128, S], F32, tag="proj", name="pr_ps")
                    for ci in range(CC):
                        nc.tensor.matmul(
                            pr_ps[:],
                            wp_sb[:, ci, co * 128:(co + 1) * 128],
                            on_sb[:, ci, qh * S:(qh + 1) * S],
                            start=(ci == 0), stop=(ci == CC - 1),
                        )
                    res = norm.tile([128, S], F32, tag="rnb", name="res")
                    nc.vector.scalar_tensor_tensor(
                        out=res[:],
                        in0=pr_ps[:],
                        scalar=bvp_sb[:, co:co + 1],
                        in1=xr[:, co, :],
                        op0=mybir.AluOpType.add,
                        op1=mybir.AluOpType.add,
                    )
                    nc.sync.dma_start(
                        out=out_d[co * 128:(co + 1) * 128, qh * S:(qh + 1) * S],
                        in_=res[:],
                    )

    nc.finalize()
    _cached["nc"] = nc
    return nc


def _prep_inputs(x, gamma, wq, bq, wk, bk, wv, bv, wp, bp):
    x = np.asarray(x, np.float32)
    X = np.ascontiguousarray(x[0].reshape(C, SEQ))
    g = (np.asarray(gamma, np.float32) * np.float32(np.sqrt(C))).astype(np.float32)
    wq = np.asarray(wq, np.float32)
    wk = np.asarray(wk, np.float32)
    wv = np.asarray(wv, np.float32)
    wp = np.asarray(wp, np.float32)
    bq = np.asarray(bq, np.float32)
    bk = np.asarray(bk, np.float32)
    bv = np.asarray(bv, np.float32)
    bp = np.asarray(bp, np.float32)
    wqT = np.ascontiguousarray((wq * g[None, :]).T)
    wkT = np.ascontiguousarray((wk * g[None, :]).T)
    wvT = np.ascontiguousarray((wv * g[None, :]).T)
    wpT = np.ascontiguousarray(wp.T)
    bvp = (bp + wp @ bv).astype(np.float32)

    common = {
        "xkv": X,
        "wqT": wqT, "wkT": wkT, "wvT": wvT, "wpT": wpT,
        "bq": np.ascontiguousarray(bq[:, None]),
        "bk": np.ascontiguousarray(bk[:, None]),
        "bvp": np.ascontiguousarray(bvp[:, None]),
    }
    in_maps = []
    for i in range(F):
        kvb = np.zeros((128, KVC), np.float32)
        for j in range(KVC):
            if j // (HW // 128) > i:
                kvb[:, j] = NEG
        m = dict(common)
        m["xq"] = np.ascontiguousarray(X[:, i * HW:(i + 1) * HW])
        m["kvb"] = kvb
        in_maps.append(m)
    return in_maps


def kernel(x, gamma, wq, bq, wk, bk, wv, bv, wp, bp, _trace=False):
    nc = _build()
    in_maps = _prep_inputs(x, gamma, wq, bq, wk, bk, wv, bv, wp, bp)
    kwargs = {}
    if _trace:
        kwargs = dict(trace=True, trace_cores=list(range(F)))
    r = run_bass_kernel_spmd(nc, in_maps, core_ids=list(range(F)), **kwargs)
    out = np.empty((1, C, F, 32, 32), np.float32)
    for i in range(F):
        out[0, :, i] = r.results[i]["out"].reshape(C, 32, 32)
    kernel._last_results = r
    return out


# revision 8
# speedup vs baseline: 1.2005x; 1.2005x over previous
"""Block-causal attention block (RMSnorm + QKV + frame-causal attention + proj)
on 8 TRN2 NeuronCores.

Sharding: sequence-parallel over the 8 frames — core i owns the 1024 queries of
frame i and processes KV blocks for frames 0..i (uniform SPMD program: all 16
half-blocks are processed on every core; future frames are killed by a
per-core additive bias of -1e30 before the exp, so they contribute exp() = 0
to both the numerator and denominator of the softmax).

Layouts are channel-first throughout ([C, seq] with C on partitions), which
makes every contraction a natural PE matmul with no transposes:
  k^T [C, kv]   = Wk' @ xn           (lhsT = wkT chunk, rhs = xn)
  v   [kv, C]   = xn^T @ Wv'^T       (lhsT = xn chunk,  rhs = wvT)
  S^T [kv, q]   = K @ Q^T            (lhsT = k^T chunk, rhs = q^T)
  O^T [C, q]    = V^T @ P^T          (lhsT = v chunk,   rhs = p^T)
  den [1, q]    = ones^T @ P^T       (lhsT = ones,      rhs = p^T)

Host-side folds: gamma*sqrt(C) into wq/wk/wv; bv through wp into the output
bias (softmax rows sum to 1); no max-subtraction in the softmax (scores here
are O(1); exp is safe and matches jax.nn.softmax exactly up to rounding).

All matmuls run in float32r (FP22) — full-rate on TRN2 with ~1e-4 accuracy.
"""

import sys

import numpy as np

sys.path.insert(0, "/opt/trn_rl_repo")

import concourse.bacc as bacc
import concourse.bass as bass  # noqa: F401
import concourse.tile as tile
from concourse import mybir
from concourse.bass_utils import run_bass_kernel_spmd

C = 512
CC = C // 128          # 4 channel chunks
F = 8                  # frames
HW = 1024              # tokens per frame
SEQ = F * HW           # 8192
S = 512                # kv columns processed per step
NSTEP = SEQ // S       # 16
Q = 1024               # queries per core (one frame)
QH = Q // S            # 2 query halves
KVC = SEQ // 128       # 64 kv 128-chunks
SCALE = 1.0 / float(np.sqrt(C))
NEG = -1.0e30

F32 = mybir.dt.float32
F32R = mybir.dt.float32r
Act = mybir.ActivationFunctionType

_cached = {}


def _build():
    if "nc" in _cached:
        return _cached["nc"]

    nc = bacc.Bacc()
    xq_d = nc.dram_tensor("xq", [C, Q], F32, kind="ExternalInput")
    xkv_d = nc.dram_tensor("xkv", [C, SEQ], F32, kind="ExternalInput")
    kvb_d = nc.dram_tensor("kvb", [128, KVC], F32, kind="ExternalInput")
    wq_d = nc.dram_tensor("wqT", [C, C], F32, kind="ExternalInput")
    wk_d = nc.dram_tensor("wkT", [C, C], F32, kind="ExternalInput")
    wv_d = nc.dram_tensor("wvT", [C, C], F32, kind="ExternalInput")
    wp_d = nc.dram_tensor("wpT", [C, C], F32, kind="ExternalInput")
    bq_d = nc.dram_tensor("bq", [C, 1], F32, kind="ExternalInput")
    bk_d = nc.dram_tensor("bk", [C, 1], F32, kind="ExternalInput")
    bvp_d = nc.dram_tensor("bvp", [C, 1], F32, kind="ExternalInput")
    out_d = nc.dram_tensor("out", [C, Q], F32, kind="ExternalOutput")

    with tile.TileContext(nc) as tc:
        with (
            tc.tile_pool(name="const", bufs=1) as const,
            tc.tile_pool(name="persist", bufs=1) as persist,
            tc.tile_pool(name="xload", bufs=2) as xload,
            tc.tile_pool(name="norm", bufs=2) as norm,
            tc.tile_pool(name="kv", bufs=2) as kvpool,
            tc.tile_pool(name="ppool", bufs=2) as ppool,
            tc.tile_pool(name="dram", bufs=1, space="DRAM") as drampool,
            tc.tile_pool(name="psum_s", bufs=2, space="PSUM") as psum_s,
            tc.tile_pool(name="psum_o", bufs=2, space="PSUM") as psum_o,
            tc.tile_pool(name="psum_den", bufs=1, space="PSUM") as psum_den,
        ):
            # ---- constants / weights (wq and wp share one slot: wp is only
            # needed after the last use of wq) ----
            wq_sb = const.tile([128, CC, C], F32R, tag="wqp", name="wq_sb")
            wk_sb = const.tile([128, CC, C], F32R, tag="wk", name="wk_sb")
            wv_sb = const.tile([128, CC, C], F32R, tag="wv", name="wv_sb")
            for w_sb, w_d in ((wq_sb, wq_d), (wk_sb, wk_d), (wv_sb, wv_d)):
                for ci in range(CC):
                    nc.sync.dma_start(
                        out=w_sb[:, ci, :],
                        in_=w_d[ci * 128:(ci + 1) * 128, :].bitcast(F32R),
                    )
            bq_sb = const.tile([128, CC], F32, tag="bq", name="bq_sb")
            bk_sb = const.tile([128, CC], F32, tag="bk", name="bk_sb")
            bvp_sb = const.tile([128, CC], F32, tag="bvp", name="bvp_sb")
            for b_sb, b_d in ((bq_sb, bq_d), (bk_sb, bk_d), (bvp_sb, bvp_d)):
                for ci in range(CC):
                    nc.sync.dma_start(
                        out=b_sb[:, ci:ci + 1],
                        in_=b_d[ci * 128:(ci + 1) * 128, :],
                    )
            kvb_sb = const.tile([128, KVC], F32, tag="kvb", name="kvb_sb")
            nc.sync.dma_start(out=kvb_sb[:], in_=kvb_d[:])
            ones_f = const.tile([128, 1], F32, tag="ones_f", name="ones_f")
            nc.vector.memset(ones_f[:], 1.0)
            ones_sb = const.tile([128, 1], F32R, tag="ones", name="ones_sb")
            nc.vector.tensor_copy(ones_sb[:], ones_f[:])

            # ---- persistent q-side tiles ----
            qT_sb = persist.tile([128, CC, Q], F32R, tag="qT", name="qT_sb")
            o_sb = persist.tile([128, CC, Q], F32, tag="o", name="o_sb")
            rdb = persist.tile([128, Q], F32, tag="rdb", name="rdb")
            den_ps = [
                psum_den.tile([1, S], F32, tag=f"den{qh}", name=f"den{qh}")
                for qh in range(QH)
            ]

            # ---- stats prepass: per-column ln(sum x^2) for q halves (u=0,1)
            # and kv steps (u=2..17), parked in DRAM; the main loop
            # broadcast-DMAs each row back and applies exp(-0.5*ln) so the
            # scalar engine only ever needs Ln and Exp tables ----
            ln_dram = drampool.tile([18, S], F32, tag="ln_dram", name="ln_dram")

            def stats_step(u, x_dram, col0):
                xt = xload.tile([128, CC, S], F32, tag="xt", name="xt")
                for ci in range(CC):
                    nc.sync.dma_start(
                        out=xt[:, ci, :],
                        in_=x_dram[ci * 128:(ci + 1) * 128, col0:col0 + S],
                    )
                xsq = norm.tile([128, CC, S], F32R, tag="xsq", name="xsq")
                ss_ps = psum_s.tile([1, S], F32, tag="s", name="ss_ps")
                for ci in range(CC):
                    nc.vector.tensor_mul(xsq[:, ci, :], xt[:, ci, :], xt[:, ci, :])
                for ci in range(CC):
                    nc.tensor.matmul(
                        ss_ps[:], ones_sb[:], xsq[:, ci, :],
                        start=(ci == 0), stop=(ci == CC - 1),
                    )
                ln_t = norm.tile([1, S], F32, tag="ln_t", name="ln_t")
                nc.scalar.activation(ln_t[:], ss_ps[:], Act.Ln)
                nc.sync.dma_start(out=ln_dram[u:u + 1, :], in_=ln_t[:])

            for qh in range(QH):
                stats_step(qh, xq_d, qh * S)
            for t in range(NSTEP):
                stats_step(2 + t, xkv_d, t * S)

            def load_xn(u, x_dram, col0):
                """Reload x slab and normalize columns: xn = x * exp(-ln/2)."""
                xt = xload.tile([128, CC, S], F32, tag="xt", name="xt")
                for ci in range(CC):
                    nc.sync.dma_start(
                        out=xt[:, ci, :],
                        in_=x_dram[ci * 128:(ci + 1) * 128, col0:col0 + S],
                    )
                lnb = norm.tile([128, S], F32, tag="lnb", name="lnb")
                nc.sync.dma_start(
                    out=lnb[:], in_=ln_dram[u:u + 1, :].to_broadcast([128, S])
                )
                rnb = norm.tile([128, S], F32, tag="rnb", name="rnb")
                nc.scalar.activation(rnb[:], lnb[:], Act.Exp, scale=-0.5)
                xn = norm.tile([128, CC, S], F32R, tag="xn", name="xn")
                for ci in range(CC):
                    nc.vector.tensor_mul(xn[:, ci, :], xt[:, ci, :], rnb[:])
                return xn

            # ---- Q path (once) ----
            for qh in range(QH):
                xn = load_xn(qh, xq_d, qh * S)
                for co in range(CC):
                    q_ps = psum_o.tile([128, S], F32, tag="proj", name="q_ps")
                    for ci in range(CC):
                        nc.tensor.matmul(
                            q_ps[:],
                            wq_sb[:, ci, co * 128:(co + 1) * 128],
                            xn[:, ci, :],
                            start=(ci == 0), stop=(ci == CC - 1),
                        )
                    nc.vector.tensor_scalar_add(
                        qT_sb[:, co, qh * S:(qh + 1) * S], q_ps[:],
                        bq_sb[:, co:co + 1],
                    )

            # wp loads into wq's slot once wq is no longer needed
            wp_sb = const.tile([128, CC, C], F32R, tag="wqp", name="wp_sb")
            for ci in range(CC):
                nc.sync.dma_start(
                    out=wp_sb[:, ci, :],
                    in_=wp_d[ci * 128:(ci + 1) * 128, :].bitcast(F32R),
                )

            # ---- KV steps ----
            for t in range(NSTEP):
                xn = load_xn(2 + t, xkv_d, t * S)

                kT = kvpool.tile([128, CC, S], F32R, tag="kT", name="kT")
                for co in range(CC):
                    k_ps = psum_o.tile([128, S], F32, tag="proj", name="k_ps")
                    for ci in range(CC):
                        nc.tensor.matmul(
                            k_ps[:],
                            wk_sb[:, ci, co * 128:(co + 1) * 128],
                            xn[:, ci, :],
                            start=(ci == 0), stop=(ci == CC - 1),
                        )
                    nc.vector.tensor_scalar_add(
                        kT[:, co, :], k_ps[:], bk_sb[:, co:co + 1],
                    )

                v_sb = kvpool.tile([128, S // 128, C], F32R, tag="v", name="v_sb")
                for kp in range(S // 128):
                    v_ps = psum_o.tile([128, C], F32, tag="proj", name="v_ps")
                    for ci in range(CC):
                        nc.tensor.matmul(
                            v_ps[:],
                            xn[:, ci, kp * 128:(kp + 1) * 128],
                            wv_sb[:, ci, :],
                            start=(ci == 0), stop=(ci == CC - 1),
                        )
                    nc.vector.tensor_copy(v_sb[:, kp, :], v_ps[:])

                p_sb = ppool.tile([128, S // 128, Q], F32R, tag="p", name="p_sb")
                for kp in range(S // 128):
                    kvi = t * (S // 128) + kp
                    for qh in range(QH):
                        s_ps = psum_s.tile([128, S], F32, tag="s", name="s_ps")
                        for ci in range(CC):
                            nc.tensor.matmul(
                                s_ps[:],
                                kT[:, ci, kp * 128:(kp + 1) * 128],
                                qT_sb[:, ci, qh * S:(qh + 1) * S],
                                start=(ci == 0), stop=(ci == CC - 1),
                            )
                        nc.scalar.activation(
                            p_sb[:, kp, qh * S:(qh + 1) * S], s_ps[:],
                            Act.Exp, bias=kvb_sb[:, kvi:kvi + 1], scale=SCALE,
                        )
                        nc.tensor.matmul(
                            den_ps[qh][:], ones_sb[:],
                            p_sb[:, kp, qh * S:(qh + 1) * S],
                            start=(t == 0 and kp == 0),
                            stop=(t == NSTEP - 1 and kp == S // 128 - 1),
                        )

                for co in range(CC):
                    for qh in range(QH):
                        o_ps = psum_o.tile([128, S], F32, tag="o", name="o_ps")
                        for kp in range(S // 128):
                            nc.tensor.matmul(
                                o_ps[:],
                                v_sb[:, kp, co * 128:(co + 1) * 128],
                                p_sb[:, kp, qh * S:(qh + 1) * S],
                                start=(kp == 0), stop=(kp == S // 128 - 1),
                            )
                        if t == 0:
                            nc.vector.tensor_copy(
                                o_sb[:, co, qh * S:(qh + 1) * S], o_ps[:]
                            )
                        else:
                            nc.vector.tensor_add(
                                o_sb[:, co, qh * S:(qh + 1) * S],
                                o_sb[:, co, qh * S:(qh + 1) * S],
                                o_ps[:],
                            )

            # ---- finalize: normalize, project, residual ----
            for qh in range(QH):
                rd = norm.tile([1, S], F32, tag="rn", name="rd")
                nc.vector.reciprocal(rd[:], den_ps[qh][:])
                nc.gpsimd.partition_broadcast(rdb[:, qh * S:(qh + 1) * S], rd[:])
            # o_n := o * (1/den), rounded to fp32r (reuses a p-pool slot)
            on_sb = ppool.tile([128, CC, Q], F32R, tag="p", name="on_sb")
            for ci in range(CC):
                nc.vector.tensor_mul(on_sb[:, ci, :], o_sb[:, ci, :], rdb[:])
            for qh in range(QH):
                xr = xload.tile([128, CC, S], F32, tag="xt", name="xr")
                for ci in range(CC):
                    nc.sync.dma_start(
                        out=xr[:, ci, :],
                        in_=xq_d[ci * 128:(ci + 1) * 128, qh * S:(qh + 1) * S],
                    )
                for co in range(CC):
                    pr_ps = psum_o.tile([128, S], F32, tag="proj", name="pr_ps")
                    for ci in range(CC):
                        nc.tensor.matmul(
                            pr_ps[:],
                            wp_sb[:, ci, co * 128:(co + 1) * 128],
                            on_sb[:, ci, qh * S:(qh + 1) * S],
                            start=(ci == 0), stop=(ci == CC - 1),
                        )
                    res = norm.tile([128, S], F32, tag="rnb", name="res")
                    nc.vector.scalar_tensor_tensor(
                        out=res[:],
                        in0=pr_ps[:],
                        scalar=bvp_sb[:, co:co + 1],
                        in1=xr[:, co, :],
                        op0=mybir.AluOpType.add,
                        op1=mybir.AluOpType.add,
                    )
                    nc.sync.dma_start(
                        out=out_d[co * 128:(co + 1) * 128, qh * S:(qh + 1) * S],
                        in_=res[:],
                    )

    nc.finalize()
    _cached["nc"] = nc
    return nc


def _prep_inputs(x, gamma, wq, bq, wk, bk, wv, bv, wp, bp):
    x = np.asarray(x, np.float32)
    X = np.ascontiguousarray(x[0].reshape(C, SEQ))
    g = (np.asarray(gamma, np.float32) * np.float32(np.sqrt(C))).astype(np.float32)
    wq = np.asarray(wq, np.float32)
    wk = np.asarray(wk, np.float32)
    wv = np.asarray(wv, np.float32)
    wp = np.asarray(wp, np.float32)
    bq = np.asarray(bq, np.float32)
    bk = np.asarray(bk, np.float32)
    bv = np.asarray(bv, np.float32)
    bp = np.asarray(bp, np.float32)
    wqT = np.ascontiguousarray((wq * g[None, :]).T)
    wkT = np.ascontiguousarray((wk * g[None, :]).T)
    wvT = np.ascontiguousarray((wv * g[None, :]).T)
    wpT = np.ascontiguousarray(wp.T)
    bvp = (bp + wp @ bv).astype(np.float32)

    common = {
        "xkv": X,
        "wqT": wqT, "wkT": wkT, "wvT": wvT, "wpT": wpT,
        "bq": np.ascontiguousarray(bq[:, None]),
        "bk": np.ascontiguousarray(bk[:, None]),
        "bvp": np.ascontiguousarray(bvp[:, None]),
    }
    in_maps = []
    for i in range(F):
        kvb = np.zeros((128, KVC), np.float32)
        for j in range(KVC):
            if j // (HW // 128) > i:
                kvb[:, j] = NEG
        m = dict(common)
        m["xq"] = np.ascontiguousarray(X[:, i * HW:(i + 1) * HW])
        m["kvb"] = kvb
        in_maps.append(m)
    return in_maps


def kernel(x, gamma, wq, bq, wk, bk, wv, bv, wp, bp, _trace=False):
    nc = _build()
    in_maps = _prep_inputs(x, gamma, wq, bq, wk, bk, wv, bv, wp, bp)
    kwargs = {}
    if _trace:
        kwargs = dict(trace=True, trace_cores=list(range(F)))
    r = run_bass_kernel_spmd(nc, in_maps, core_ids=list(range(F)), **kwargs)
    out = np.empty((1, C, F, 32, 32), np.float32)
    for i in range(F):
        out[0, :, i] = r.results[i]["out"].reshape(C, 32, 32)
    kernel._last_results = r
    return out
